# revision 11
# baseline (speedup 1.0000x reference)
"""GCN (2 dense + 3 sparse layers + log_softmax) on 8 Trainium2 NeuronCores.

Strategy: each graph aggregation A_norm @ H runs densely on the PE as
out_T[f, t] = sum_s H'[s, f] * B^T[s, t] with B the count-valued adjacency in
fp8 (exact small integers) streamed as the MOVING operand in DoubleRow perf
mode (2 fp8 elements per cell per cycle -> 2 moving columns/cycle), with the
diag-scaled features H' quantized to fp8e4m3 as the stationary operand.
Source chunks are processed in PAIRS (virtual contraction dim 256), so one
pass over the [12288 x 1536] per-core adjacency block costs 48*3 matmuls of
FD=1024 (~31us at 2 cols/cycle) independent of feature width - layer 4
(d=128) runs in a single pass.

Nodes (dst) are row-sharded 8 ways; each layer's feature block is exchanged
via AllGather in fp8 in thirds aligned to the psum column tiles, so each
third's collective pipelines under compute.  The two adjacency matrices are
RESIDENT in SBUF: bden streams once (l1+l2), bsp overwrites it in REVERSE
group order while l2/l3 consume their pairs in reverse stream order - so the
bsp stream starts the moment l2 starts and l3 never waits on it.
"""

import os
import numpy as np
import ml_dtypes

import concourse.bacc as bacc
import concourse.mybir as mybir
import concourse.tile as tile
from concourse.bass_utils import run_bass_kernel_spmd

# ---- problem constants ----
N = 12000
NP = 12288         # padded nodes (96 * 128)
NCORES = 8
NLOC = NP // NCORES            # 1536 rows per core
KC = NP // 128                 # 96 k-chunks
MC = NLOC // 128               # 12 local row chunks
MT = 4                         # m-chunks per third (psum col tile)
NT = NLOC // 512               # 3 psum col tiles / exchange thirds
NGRP = 8                       # B-stream DMA groups
GC = KC // NGRP                # 12 chunks per group
GW = GC * NLOC                 # group width in cden columns
F_IN = 512
CLS = 6

F8 = mybir.dt.float8e4
F16 = mybir.dt.float16
F32 = mybir.dt.float32
NP_F8 = ml_dtypes.float8_e4m3
NP_F16 = np.float16

D1, D2, D3, D4, D5 = 32, 32, 64, 128, 32   # aggregation widths per layer

# B consumption order: thirds (psum col tiles of the producing layer) outer,
# so each third's exchange covers a contiguous block of stream positions;
# B matrices are stored in this order host-side -> sequential DMA stream.
# Stream-position pairs (2p, 2p+1) must be globally adjacent chunks for the
# DoubleRow stationary-operand pair view of h_full.
CHUNK_ORDER = [c * MC + T * MT + j
               for T in range(NT) for c in range(NCORES) for j in range(MT)]
for _p in range(KC // 2):
    assert CHUNK_ORDER[2 * _p + 1] == CHUNK_ORDER[2 * _p] + 1

_cached = {}


def _build_program():
    nc = bacc.Bacc("TRN2", target_bir_lowering=False, debug=False,
                   num_devices=NCORES)

    bden = nc.dram_tensor("bden", [NGRP, 128, GW], F8, kind="ExternalInput")
    bsp = nc.dram_tensor("bsp", [NGRP, 128, GW], F8, kind="ExternalInput")
    featT = nc.dram_tensor("featT", [4, 128, NLOC], F16, kind="ExternalInput")
    w1 = nc.dram_tensor("w1", [4, 128, 32], F16, kind="ExternalInput")
    w12b = nc.dram_tensor("w12b", [33, 64], F16, kind="ExternalInput")
    w13b = nc.dram_tensor("w13b", [65, 128], F16, kind="ExternalInput")
    w14 = nc.dram_tensor("w14", [128, 128], F16, kind="ExternalInput")
    w2 = nc.dram_tensor("w2", [128, CLS], F16, kind="ExternalInput")
    biases_pp = nc.dram_tensor("biases_pp", [128, 3], F32, kind="ExternalInput")
    dis_repl = nc.dram_tensor("dis_repl", [128, NLOC], F16, kind="ExternalInput")
    dinv_repl = nc.dram_tensor("dinv_repl", [128, NLOC], F16, kind="ExternalInput")
    dis_pp = nc.dram_tensor("dis_pp", [128, MC], F32, kind="ExternalInput")
    dinv_pp = nc.dram_tensor("dinv_pp", [128, MC], F32, kind="ExternalInput")
    ident16 = nc.dram_tensor("ident16", [128, 128], F16, kind="ExternalInput")
    ident32 = nc.dram_tensor("ident32", [128, 128], F32, kind="ExternalInput")
    out = nc.dram_tensor("out", [128, MC * CLS], F32, kind="ExternalOutput")

    AG = mybir.AluOpType
    AF = mybir.ActivationFunctionType
    DR = mybir.MatmulPerfMode.DoubleRow
    RG = [list(range(NCORES))]

    with tile.TileContext(nc) as tc:
        with (
            tc.tile_pool(name="const", bufs=1) as cpool,
            tc.tile_pool(name="dscale", bufs=1) as dpool_s,
            tc.tile_pool(name="resident", bufs=1) as hpool,
            tc.tile_pool(name="fq", bufs=2) as fpool,
            tc.tile_pool(name="work", bufs=1) as wpool,
            tc.tile_pool(name="post", bufs=3) as ppool,
            tc.tile_pool(name="small", bufs=2) as spool,
            tc.tile_pool(name="agg", bufs=4, space="PSUM") as aggp,
            tc.tile_pool(name="wmm", bufs=2, space="PSUM") as wmmp,
            tc.tile_pool(name="tp", bufs=1, space="PSUM") as tpp,
            tc.tile_pool(name="dram", bufs=1, space="DRAM") as dpool,
        ):
            # ---------- early constants (l1 path only) ----------
            w1_sb = cpool.tile([128, 4 * 32], F16, tag="w1")
            nc.scalar.dma_start(w1_sb[:].rearrange("p (c j) -> p c j", c=4),
                                w1.ap().rearrange("c p j -> p c j"))
            bias_sb = cpool.tile([128, 3], F32, tag="bias")
            nc.scalar.dma_start(bias_sb[:], biases_pp[:, :])
            dispp_sb = cpool.tile([128, MC], F32, tag="dispp")
            nc.scalar.dma_start(dispp_sb[:], dis_pp[:, :])
            id16_sb = cpool.tile([128, 128], F16, tag="id16")
            nc.scalar.dma_start(id16_sb[:], ident16[:, :])

            # resident adjacency (bden first, overwritten by bsp during l2)
            cden = hpool.tile([128, KC * NLOC], F8, tag="cden")
            cv = cden[:].rearrange("p (i t) -> p i t", i=KC)
            # gathered features, ping-pong so fills for layer k+1 never
            # touch the buffer layer k's pass is reading:
            #   A: l1 (32), l3 (64), l5 (32);  B: l2 (32), l4 (128)
            h_fullA = hpool.tile([128, KC * 64], F8, tag="hfullA")
            h_fullB = hpool.tile([128, KC * 128], F8, tag="hfullB")
            h_for = {"l1": h_fullA, "l2": h_fullB, "l3": h_fullA,
                     "l4": h_fullB, "l5": h_fullA}

            # featT quarters FIRST on the sync queue (only 1.5 MB ahead of
            # the bden stream; the l1 transform is the prologue critical path)
            fqs = []
            for kc in range(4):
                fq = fpool.tile([128, NLOC], F16, tag="fq", name=f"fq{kc}")
                nc.sync.dma_start(fq[:], featT[kc, :, :])
                fqs.append(fq)

            # bden stream; l1 consumes groups as they land
            bden_gs = []
            for g in range(NGRP):
                bden_gs.append(nc.sync.dma_start(
                    cden[:, g * GW:(g + 1) * GW], bden[g, :, :]))

            bins = {}
            fills = {}

            def m_exchange(hloc, d, lname, T):
                """AllGather third T of the local block and scatter it into
                the layer's h buffer (chunk c*MC+4T+j gets d cols)."""
                w = MT * d
                bin_t = dpool.tile([128, w], F8, tag=f"agi{lname}{T}")
                bout_t = dpool.tile([NCORES, 128, w], F8,
                                    tag=f"ago{lname}{T}",
                                    addr_space="Shared")
                bi = nc.scalar.dma_start(bin_t[:],
                                         hloc[:, T * w:(T + 1) * w])
                nc.gpsimd.collective_compute(
                    "AllGather", AG.bypass, replica_groups=RG,
                    ins=[bin_t.opt()], outs=[bout_t.opt()],
                )
                hf = h_for[lname]
                view = hf[:, 0:KC * d].rearrange("p (c x) -> p c x",
                                                 c=NCORES)
                fi = nc.sync.dma_start(
                    view[:, :, T * w:(T + 1) * w],
                    bout_t[:, :, :].rearrange("c p w -> p c w"))
                bins[(lname, T)] = bi
                fills[(lname, T)] = fi

            def pair_mm(agg, d, lname, p, t, start, stop):
                k0 = CHUNK_ORDER[2 * p]
                lhs = h_for[lname][:, k0 * d:(k0 + 2) * d].rearrange(
                    "p (k f) -> p k f", k=2)
                return nc.tensor.matmul(
                    agg[:, :], lhs,
                    cv[:, 2 * p:2 * p + 2, t * 512:(t + 1) * 512],
                    start=start, stop=stop, perf_mode=DR)

            def bpass_s(d, lname, rev=False):
                """Stream-ordered pass (pairs outer, psum tiles inner):
                adjacency groups are consumed & freed progressively, in
                stream order or reversed."""
                aggs = [aggp.tile([d, 512], F32, tag="agg",
                                  name=f"agg_{lname}_{i}") for i in range(NT)]
                ps = range(KC // 2 - 1, -1, -1) if rev else range(KC // 2)
                for i, p in enumerate(ps):
                    for t in range(NT):
                        pair_mm(aggs[t], d, lname, p, t,
                                start=(i == 0), stop=(i == KC // 2 - 1))
                return aggs

            def bpass_t(d, lname):
                """Tile-sequential pass: yields each completed psum col tile
                so its epilogue + exchange pipeline under the next tiles.
                Sections are chained so the scheduler cannot interleave
                them."""
                prev = [None]
                for t in range(NT):
                    agg = aggp.tile([d, 512], F32, tag="agg",
                                    name=f"agg_{lname}_{t}")
                    for p in range(KC // 2):
                        mm = pair_mm(agg, d, lname, p, t,
                                     start=(p == 0), stop=(p == KC // 2 - 1))
                        if p == 0 and prev[0] is not None:
                            tile.add_dep_helper(mm.ins, prev[0].ins,
                                                sync=True,
                                                reason=f"chain {lname} s{t}")
                    prev[0] = mm
                    yield t, agg

            # ============ L1 local transform: H'1 = dis * (X0 @ W1) ==========
            h1loc = wpool.tile([128, MC * D1], F8, tag="h1loc")
            t1k = []
            for kc in range(4):
                tk = aggp.tile([128, MC * 32], F32, tag="agg", name=f"t1k{kc}")
                t1k.append(tk)
                for m in range(MC):
                    nc.tensor.matmul(
                        tk[:, m * 32:(m + 1) * 32],
                        fqs[kc][:, m * 128:(m + 1) * 128],
                        w1_sb[:, kc * 32:(kc + 1) * 32],
                        start=True, stop=True,
                    )
            for T in range(NT):
                cols = slice(T * MT * 32, (T + 1) * MT * 32)
                s01 = spool.tile([128, MT * 32], F32, tag="t1s",
                                 name=f"s01_{T}")
                nc.vector.tensor_copy(s01[:, :], t1k[0][:, cols])
                for kc in range(1, 4):
                    nc.vector.tensor_tensor(s01[:, :], s01[:, :],
                                            t1k[kc][:, cols], op=AG.add)
                for m in range(T * MT, (T + 1) * MT):
                    col = (m % MT) * 32
                    nc.vector.tensor_scalar_mul(
                        h1loc[:, m * D1:(m + 1) * D1],
                        s01[:, col:col + 32],
                        dispp_sb[:, m:m + 1])
                m_exchange(h1loc, D1, "l1", T)
            tile.add_dep_helper(bden_gs[4].ins, bins[("l1", 2)].ins,
                                sync=True,
                                reason="l1 bins before late bden groups")

            # dis/dinv replicated scale rows (loaded in the quiet window
            # before the bsp stream starts)
            disr_sb = dpool_s.tile([128, NLOC], F16, tag="dsc", name="disr")
            nc.scalar.dma_start(disr_sb[:], dis_repl[:, :])
            dinvr_sb = dpool_s.tile([128, NLOC], F16, tag="dinv", name="dinvr")
            nc.scalar.dma_start(dinvr_sb[:], dinv_repl[:, :])

            # ============ L1 agg + post: x1 = relu(dis*G1 + b1) ==============
            # epilogue + l2-input exchange in REVERSE thirds: l2 consumes its
            # pairs in reverse stream order (so the bsp stream, written in
            # reverse group order, can start as soon as l2 starts).
            aggs = bpass_s(D1, "l1")
            x1p = ppool.tile([32, NLOC], F16, tag="post", name="x1p")
            h2loc = wpool.tile([128, MC * D2], F8, tag="h2loc")
            tp1 = tpp.tile([128, MC * 32], F16, tag="tp16")
            for T in (2, 1, 0):
                sl = slice(T * 512, (T + 1) * 512)
                g1s = spool.tile([32, 512], F32, tag="g1s", name=f"g1s_{T}")
                nc.vector.tensor_tensor(
                    g1s[:, :], aggs[T][:, :], disr_sb[0:32, sl], op=AG.mult)
                x1t = spool.tile([32, 512], F16, tag="x1t", name=f"x1t_{T}")
                nc.scalar.activation(x1t[:, :], g1s[:, :], AF.Relu,
                                     bias=bias_sb[0:32, 0:1])
                nc.vector.tensor_tensor(
                    x1p[:, sl], x1t[:, :], disr_sb[0:32, sl], op=AG.mult)
                for m in range(T * MT, (T + 1) * MT):
                    nc.tensor.transpose(
                        tp1[:, m * 32:(m + 1) * 32],
                        x1p[:, m * 128:(m + 1) * 128], id16_sb[0:32, 0:32])
                o = T * MT * D2
                nc.vector.tensor_copy(h2loc[:, o:o + MT * D2],
                                      tp1[:, o:o + MT * D2])
                m_exchange(h2loc, D2, "l2", T)

            # ============ L2: agg + x2 = relu(dis*G2 @ W12 + b12) ============
            w12_sb = cpool.tile([33, 64], F16, tag="w12")
            nc.scalar.dma_start(w12_sb[:], w12b[:, :])
            dinvpp_sb = cpool.tile([128, MC], F32, tag="dinvpp")
            nc.scalar.dma_start(dinvpp_sb[:], dinv_pp[:, :])
            aggs = bpass_s(D2, "l2", rev=True)
            # bsp overwrites cden groups in REVERSE order as l2's reversed
            # pass frees them.  The last two groups (g1, g0) additionally
            # wait for the l3 fill so the l3 exchange gets the fabric to
            # itself; l3's reversed pass needs g0 last.
            bsp_gs = {}
            for g in range(NGRP - 1, -1, -1):
                bsp_gs[g] = nc.sync.dma_start(
                    cden[:, g * GW:(g + 1) * GW], bsp[g, :, :])
            g2p = ppool.tile([33, NLOC], F16, tag="post", name="g2p")
            nc.vector.memset(g2p[32:33, :], 1.0)
            h3loc = wpool.tile([128, MC * D3], F8, tag="h3loc")
            for T in (2, 1, 0):
                sl = slice(T * 512, (T + 1) * 512)
                nc.vector.tensor_tensor(
                    g2p[0:32, sl], aggs[T][:, :], disr_sb[0:32, sl],
                    op=AG.mult)
                for m in range(T * MT, (T + 1) * MT):
                    xp = wmmp.tile([128, 64], F32, tag="wmm", name=f"x2_{m}")
                    nc.tensor.matmul(xp[:, :], g2p[:, m * 128:(m + 1) * 128],
                                     w12_sb[:, :], start=True, stop=True)
                    nc.vector.tensor_scalar(
                        h3loc[:, m * D3:(m + 1) * D3], xp[:, :],
                        0.0, dinvpp_sb[:, m:m + 1], op0=AG.max, op1=AG.mult)
                m_exchange(h3loc, D3, "l3", T)
            for g in (1, 0):
                tile.add_dep_helper(bsp_gs[g].ins, fills[("l3", 0)].ins,
                                    sync=True,
                                    reason="l3 exchange before last bsp")

            # ============ L3: agg + x3 = relu(dinv*G3 @ W13 + b13) ===========
            w13_sb = cpool.tile([65, 128], F16, tag="w13")
            nc.scalar.dma_start(w13_sb[:], w13b[:, :])
            aggs = bpass_s(D3, "l3", rev=True)
            g3p = ppool.tile([65, NLOC], F16, tag="post", name="g3p")
            nc.vector.memset(g3p[64:65, :], 1.0)
            h4loc = wpool.tile([128, MC * D4], F8, tag="h4loc")
            for T in range(NT):
                sl = slice(T * 512, (T + 1) * 512)
                nc.vector.tensor_tensor(
                    g3p[0:64, sl], aggs[T][:, :], dinvr_sb[0:64, sl],
                    op=AG.mult)
                for m in range(T * MT, (T + 1) * MT):
                    xp = wmmp.tile([128, 128], F32, tag="wmm", name=f"x3_{m}")
                    nc.tensor.matmul(xp[:, :], g3p[:, m * 128:(m + 1) * 128],
                                     w13_sb[:, :], start=True, stop=True)
                    nc.vector.tensor_scalar(
                        h4loc[:, m * D4:(m + 1) * D4], xp[:, :],
                        0.0, dinvpp_sb[:, m:m + 1], op0=AG.max, op1=AG.mult)
                m_exchange(h4loc, D4, "l4", T)

            # ===== L4: agg + x4T = relu(W14^T @ (dinv*G4) + b14) =============
            # ===== L5a: H'5T = dinv * (W2^T @ x4T), transpose, exchange ======
            w14_sb = cpool.tile([128, 128], F16, tag="w14")
            nc.scalar.dma_start(w14_sb[:], w14[:, :])
            w2_sb = cpool.tile([128, CLS], F16, tag="w2")
            nc.scalar.dma_start(w2_sb[:], w2[:, :])
            g4p = ppool.tile([128, NLOC], F16, tag="post", name="g4p")
            x4T = ppool.tile([128, NLOC], F16, tag="post", name="x4T")
            h5T = ppool.tile([32, NLOC], F16, tag="post", name="h5T")
            nc.vector.memset(h5T[0:32, :], 0.0)
            h5loc = wpool.tile([128, MC * D5], F8, tag="h5loc")
            tp5 = tpp.tile([128, MC * 32], F16, tag="tp16")
            for T, agg in bpass_t(D4, "l4"):
                sl = slice(T * 512, (T + 1) * 512)
                nc.vector.tensor_tensor(
                    g4p[:, sl], agg[:, :], dinvr_sb[:, sl], op=AG.mult)
                x4p = wmmp.tile([128, 512], F32, tag="wmm", name=f"x4_{T}")
                nc.tensor.matmul(x4p[:, :], w14_sb[:, :], g4p[:, sl],
                                 start=True, stop=True)
                nc.scalar.activation(x4T[:, sl], x4p[:, :], AF.Relu,
                                     bias=bias_sb[:, 1:2])
                t5 = wmmp.tile([CLS, 512], F32, tag="wmm", name=f"t5_{T}")
                nc.tensor.matmul(t5[:, :], w2_sb[:, :], x4T[:, sl],
                                 start=True, stop=True)
                nc.vector.tensor_tensor(
                    h5T[0:CLS, sl], t5[:, :], dinvr_sb[0:CLS, sl],
                    op=AG.mult)
                for m in range(T * MT, (T + 1) * MT):
                    nc.tensor.transpose(
                        tp5[:, m * 32:(m + 1) * 32],
                        h5T[:, m * 128:(m + 1) * 128], id16_sb[0:32, 0:32])
                o = T * MT * D5
                nc.vector.tensor_copy(h5loc[:, o:o + MT * D5],
                                      tp5[:, o:o + MT * D5])
                m_exchange(h5loc, D5, "l5", T)

            # ============ L5b: agg + z = dinv*G5 + b2, log_softmax ===========
            id32_sb = cpool.tile([128, 128], F32, tag="id32")
            nc.scalar.dma_start(id32_sb[:], ident32[:, :])
            zt = wpool.tile([32, NLOC], F32, tag="zt")
            nc.vector.memset(zt[0:32, :], 0.0)
            ztp = tpp.tile([128, MC * 32], F32, tag="tp32")
            outsb = wpool.tile([128, MC * CLS], F32, tag="outsb")
            nmt = wpool.tile([128, MC], F32, tag="nmt")
            et = wpool.tile([128, MC * CLS], F32, tag="et")
            st = wpool.tile([128, MC], F32, tag="st")
            lst = wpool.tile([128, MC], F32, tag="lst")
            zs = wpool.tile([128, MC * CLS], F32, tag="zs")
            ztpv = ztp[:].rearrange("p (m f) -> p m f", m=MC)
            etv = et[:].rearrange("p (m f) -> p m f", m=MC)
            for T, agg in bpass_t(D5, "l5"):
                sl = slice(T * 512, (T + 1) * 512)
                ms = slice(T * MT, (T + 1) * MT)
                nc.vector.tensor_tensor(
                    zt[0:CLS, sl], agg[0:CLS, :], dinvr_sb[0:CLS, sl],
                    op=AG.mult)
                nc.vector.tensor_scalar_add(
                    zt[0:CLS, sl], zt[0:CLS, sl], bias_sb[0:CLS, 2:3])
                for m in range(T * MT, (T + 1) * MT):
                    nc.tensor.transpose(
                        ztp[:, m * 32:(m + 1) * 32],
                        zt[:, m * 128:(m + 1) * 128], id32_sb[0:32, 0:32])
                nc.vector.reduce_max(
                    nmt[:, ms], ztpv[:, ms, 0:CLS],
                    axis=mybir.AxisListType.X, negate=True)
                for m in range(T * MT, (T + 1) * MT):
                    nc.vector.tensor_scalar_add(
                        zs[:, m * CLS:(m + 1) * CLS],
                        ztp[:, m * 32: m * 32 + CLS], nmt[:, m:m + 1])
                nc.scalar.activation(et[:, T * MT * CLS:(T + 1) * MT * CLS],
                                     zs[:, T * MT * CLS:(T + 1) * MT * CLS],
                                     AF.Exp)
                nc.vector.reduce_sum(
                    st[:, ms], etv[:, ms, :], axis=mybir.AxisListType.X)
                nc.scalar.activation(lst[:, ms], st[:, ms], AF.Ln)
                for m in range(T * MT, (T + 1) * MT):
                    nc.vector.tensor_scalar(
                        outsb[:, m * CLS:(m + 1) * CLS],
                        ztp[:, m * 32: m * 32 + CLS],
                        nmt[:, m:m + 1], lst[:, m:m + 1],
                        op0=AG.add, op1=AG.subtract)
                nc.scalar.dma_start(
                    out.ap()[:, T * MT * CLS:(T + 1) * MT * CLS],
                    outsb[:, T * MT * CLS:(T + 1) * MT * CLS])

    nc.compile()
    return nc


# ---------------------------------------------------------------------------
# host-side preprocessing
# ---------------------------------------------------------------------------

def _preprocess(node_feats, edge_index, W1, b1, W12, b12, W13, b13, W14, b14,
                W2, b2):
    src = np.asarray(edge_index[0], dtype=np.int64)
    dst = np.asarray(edge_index[1], dtype=np.int64)

    # dense-path matrix: B[i,j] = #edges(i->j) offdiag, diag forced to 1
    Bden = np.zeros(NP * NP, dtype=np.uint8)
    np.add.at(Bden, src * NP + dst, 1)
    Bden = Bden.reshape(NP, NP)
    idx = np.arange(N)
    Bden[idx, idx] = 1
    deg_den = Bden[:N].sum(axis=1, dtype=np.int64).astype(np.float64)
    dis = np.zeros(NP, dtype=np.float64)
    dis[:N] = np.maximum(deg_den, 1.0) ** -0.5
    dis[N:] = 1.0

    # sparse-path matrix: Bsp[t,s] = #edges(s->t) + I
    Bsp = np.zeros(NP * NP, dtype=np.uint8)
    np.add.at(Bsp, dst * NP + src, 1)
    Bsp = Bsp.reshape(NP, NP)
    Bsp[idx, idx] += 1
    deg_sp = Bsp[:N].sum(axis=1, dtype=np.int64).astype(np.float64)
    dinv = np.zeros(NP, dtype=np.float64)
    dinv[:N] = np.where(deg_sp > 0, deg_sp.astype(np.float64) ** -0.5, 0.0)

    x0 = np.zeros((NP, F_IN), dtype=np.float32)
    x0[:N] = np.asarray(node_feats, dtype=np.float32)

    def pp(vec, c):
        loc = vec[c * NLOC:(c + 1) * NLOC].astype(np.float32)
        return np.ascontiguousarray(loc.reshape(MC, 128).T)

    def repl(vec, c):
        loc = vec[c * NLOC:(c + 1) * NLOC].astype(NP_F16)
        return np.ascontiguousarray(np.broadcast_to(loc[None, :], (128, NLOC)))

    def pack_b(B, rows):
        # [s, t_local] chunked over s, in CHUNK_ORDER, group-major so each
        # DMA group is one fully contiguous block of DRAM.
        bt = B[rows].T.reshape(KC, 128, NLOC)[CHUNK_ORDER]
        bt = bt.reshape(NGRP, GC, 128, NLOC).transpose(0, 2, 1, 3)
        return np.ascontiguousarray(bt.reshape(NGRP, 128, GW)).astype(NP_F8)

    w12b = np.concatenate([np.asarray(W12, np.float32),
                           np.asarray(b12, np.float32)[None, :]], axis=0)
    w13b = np.concatenate([np.asarray(W13, np.float32),
                           np.asarray(b13, np.float32)[None, :]], axis=0)
    biases_pp = np.zeros((128, 3), dtype=np.float32)
    biases_pp[:32, 0] = np.asarray(b1, np.float32)
    biases_pp[:, 1] = np.asarray(b14, np.float32)
    biases_pp[:CLS, 2] = np.asarray(b2, np.float32)

    in_maps = []
    for c in range(NCORES):
        rows = slice(c * NLOC, (c + 1) * NLOC)
        featT_c = np.ascontiguousarray(x0[rows].T).reshape(4, 128, NLOC)
        in_maps.append({
            "bden": pack_b(Bden, rows),
            "bsp": pack_b(Bsp, rows),
            "featT": featT_c.astype(NP_F16),
            "w1": np.asarray(W1, np.float32).reshape(4, 128, 32).astype(NP_F16),
            "w12b": w12b.astype(NP_F16),
            "w13b": w13b.astype(NP_F16),
            "w14": np.asarray(W14, np.float32).astype(NP_F16),
            "w2": np.asarray(W2, np.float32).astype(NP_F16),
            "biases_pp": biases_pp,
            "dis_repl": repl(dis, c),
            "dinv_repl": repl(dinv, c),
            "dis_pp": pp(dis, c),
            "dinv_pp": pp(dinv, c),
            "ident16": np.eye(128, dtype=NP_F16),
            "ident32": np.eye(128, dtype=np.float32),
        })
    return in_maps


def kernel(node_feats, edge_index, W1, b1, W12, b12, W13, b13, W14, b14, W2,
           b2):
    in_maps = _preprocess(node_feats, edge_index, W1, b1, W12, b12, W13, b13,
                          W14, b14, W2, b2)
    if "nc" not in _cached:
        _cached["nc"] = _build_program()
    nc = _cached["nc"]
    trace = bool(int(os.environ.get("KERNEL_TRACE", "0")))
    res = run_bass_kernel_spmd(nc, in_maps, core_ids=list(range(NCORES)),
                               trace=trace)
    _cached["last_result"] = res
    outs = []
    for c in range(NCORES):
        o = np.asarray(res.results[c]["out"])     # [128, MC*CLS]
        outs.append(o.reshape(128, MC, CLS).transpose(1, 0, 2)
                     .reshape(NLOC, CLS))
    return np.concatenate(outs, axis=0)[:N].astype(np.float32)


# revision 12
# speedup vs baseline: 1.0109x; 1.0109x over previous
"""GCN (2 dense + 3 sparse layers + log_softmax) on 8 Trainium2 NeuronCores.

Strategy: each graph aggregation A_norm @ H runs densely on the PE as
out_T[f, t] = sum_s H'[s, f] * B^T[s, t] with B the count-valued adjacency in
fp8 (exact small integers) streamed as the MOVING operand in DoubleRow perf
mode (2 fp8 elements per cell per cycle -> 2 moving columns/cycle), with the
diag-scaled features H' quantized to fp8e4m3 as the stationary operand.
Source chunks are processed in PAIRS (virtual contraction dim 256), so one
pass over the [12288 x 1536] per-core adjacency block costs 48*3 matmuls of
FD=1024 (~31us at 2 cols/cycle) independent of feature width - layer 4
(d=128) runs in a single pass.

Nodes (dst) are row-sharded 8 ways; each layer's feature block is exchanged
via AllGather in fp8 in thirds aligned to the psum column tiles, so each
third's collective pipelines under compute.  The two adjacency matrices are
RESIDENT in SBUF: bden streams once (l1+l2), bsp overwrites it in REVERSE
group order while l2/l3 consume their pairs in reverse stream order - so the
bsp stream starts the moment l2 starts and l3 never waits on it.
"""

import os
import numpy as np
import ml_dtypes

import concourse.bacc as bacc
import concourse.mybir as mybir
import concourse.tile as tile
from concourse.bass_utils import run_bass_kernel_spmd

# ---- problem constants ----
N = 12000
NP = 12288         # padded nodes (96 * 128)
NCORES = 8
NLOC = NP // NCORES            # 1536 rows per core
KC = NP // 128                 # 96 k-chunks
MC = NLOC // 128               # 12 local row chunks
MT = 4                         # m-chunks per third (psum col tile)
NT = NLOC // 512               # 3 psum col tiles / exchange thirds
NGRP = 8                       # B-stream DMA groups
GC = KC // NGRP                # 12 chunks per group
GW = GC * NLOC                 # group width in cden columns
F_IN = 512
CLS = 6

F8 = mybir.dt.float8e4
F16 = mybir.dt.float16
F32 = mybir.dt.float32
NP_F8 = ml_dtypes.float8_e4m3
NP_F16 = np.float16

D1, D2, D3, D4, D5 = 32, 32, 64, 128, 32   # aggregation widths per layer

# B consumption order: thirds (psum col tiles of the producing layer) outer,
# so each third's exchange covers a contiguous block of stream positions;
# B matrices are stored in this order host-side -> sequential DMA stream.
# Stream-position pairs (2p, 2p+1) must be globally adjacent chunks for the
# DoubleRow stationary-operand pair view of h_full.
CHUNK_ORDER = [c * MC + T * MT + j
               for T in range(NT) for c in range(NCORES) for j in range(MT)]
for _p in range(KC // 2):
    assert CHUNK_ORDER[2 * _p + 1] == CHUNK_ORDER[2 * _p] + 1

_cached = {}


def _build_program():
    nc = bacc.Bacc("TRN2", target_bir_lowering=False, debug=False,
                   num_devices=NCORES)

    bden = nc.dram_tensor("bden", [NGRP, 128, GW], F8, kind="ExternalInput")
    bsp = nc.dram_tensor("bsp", [NGRP, 128, GW], F8, kind="ExternalInput")
    featT = nc.dram_tensor("featT", [4, 128, NLOC], F16, kind="ExternalInput")
    w1 = nc.dram_tensor("w1", [4, 128, 32], F16, kind="ExternalInput")
    w12b = nc.dram_tensor("w12b", [33, 64], F16, kind="ExternalInput")
    w13b = nc.dram_tensor("w13b", [65, 128], F16, kind="ExternalInput")
    w14 = nc.dram_tensor("w14", [128, 128], F16, kind="ExternalInput")
    w2 = nc.dram_tensor("w2", [128, CLS], F16, kind="ExternalInput")
    biases_pp = nc.dram_tensor("biases_pp", [128, 3], F32, kind="ExternalInput")
    dis_repl = nc.dram_tensor("dis_repl", [128, NLOC], F16, kind="ExternalInput")
    dinv_repl = nc.dram_tensor("dinv_repl", [128, NLOC], F16, kind="ExternalInput")
    dis_pp = nc.dram_tensor("dis_pp", [128, MC], F32, kind="ExternalInput")
    dinv_pp = nc.dram_tensor("dinv_pp", [128, MC], F32, kind="ExternalInput")
    ident16 = nc.dram_tensor("ident16", [128, 128], F16, kind="ExternalInput")
    ident32 = nc.dram_tensor("ident32", [128, 128], F32, kind="ExternalInput")
    out = nc.dram_tensor("out", [128, MC * CLS], F32, kind="ExternalOutput")

    AG = mybir.AluOpType
    AF = mybir.ActivationFunctionType
    DR = mybir.MatmulPerfMode.DoubleRow
    RG = [list(range(NCORES))]

    with tile.TileContext(nc) as tc:
        with (
            tc.tile_pool(name="const", bufs=1) as cpool,
            tc.tile_pool(name="dscale", bufs=1) as dpool_s,
            tc.tile_pool(name="resident", bufs=1) as hpool,
            tc.tile_pool(name="fq", bufs=2) as fpool,
            tc.tile_pool(name="work", bufs=1) as wpool,
            tc.tile_pool(name="post", bufs=3) as ppool,
            tc.tile_pool(name="small", bufs=2) as spool,
            tc.tile_pool(name="agg", bufs=4, space="PSUM") as aggp,
            tc.tile_pool(name="wmm", bufs=2, space="PSUM") as wmmp,
            tc.tile_pool(name="tp", bufs=1, space="PSUM") as tpp,
            tc.tile_pool(name="dram", bufs=1, space="DRAM") as dpool,
        ):
            # ---------- early constants (l1 path only) ----------
            w1_sb = cpool.tile([128, 4 * 32], F16, tag="w1")
            nc.scalar.dma_start(w1_sb[:].rearrange("p (c j) -> p c j", c=4),
                                w1.ap().rearrange("c p j -> p c j"))
            bias_sb = cpool.tile([128, 3], F32, tag="bias")
            nc.scalar.dma_start(bias_sb[:], biases_pp[:, :])
            dispp_sb = cpool.tile([128, MC], F32, tag="dispp")
            nc.scalar.dma_start(dispp_sb[:], dis_pp[:, :])
            id16_sb = cpool.tile([128, 128], F16, tag="id16")
            nc.scalar.dma_start(id16_sb[:], ident16[:, :])

            # resident adjacency (bden first, overwritten by bsp during l2)
            cden = hpool.tile([128, KC * NLOC], F8, tag="cden")
            cv = cden[:].rearrange("p (i t) -> p i t", i=KC)
            # gathered features, ping-pong so fills for layer k+1 never
            # touch the buffer layer k's pass is reading:
            #   A: l1 (32), l3 (64), l5 (32);  B: l2 (32), l4 (128)
            h_fullA = hpool.tile([128, KC * 64], F8, tag="hfullA")
            h_fullB = hpool.tile([128, KC * 128], F8, tag="hfullB")
            h_for = {"l1": h_fullA, "l2": h_fullB, "l3": h_fullA,
                     "l4": h_fullB, "l5": h_fullA}

            # featT quarters FIRST on the sync queue (only 1.5 MB ahead of
            # the bden stream; the l1 transform is the prologue critical path)
            fqs = []
            for kc in range(4):
                fq = fpool.tile([128, NLOC], F16, tag="fq", name=f"fq{kc}")
                nc.sync.dma_start(fq[:], featT[kc, :, :])
                fqs.append(fq)

            # bden stream; l1 consumes groups as they land
            bden_gs = []
            for g in range(NGRP):
                bden_gs.append(nc.sync.dma_start(
                    cden[:, g * GW:(g + 1) * GW], bden[g, :, :]))

            bins = {}
            fills = {}

            def m_exchange(hloc, d, lname, T):
                """AllGather third T of the local block and scatter it into
                the layer's h buffer (chunk c*MC+4T+j gets d cols)."""
                w = MT * d
                bin_t = dpool.tile([128, w], F8, tag=f"agi{lname}{T}")
                bout_t = dpool.tile([NCORES, 128, w], F8,
                                    tag=f"ago{lname}{T}",
                                    addr_space="Shared")
                bi = nc.scalar.dma_start(bin_t[:],
                                         hloc[:, T * w:(T + 1) * w])
                nc.gpsimd.collective_compute(
                    "AllGather", AG.bypass, replica_groups=RG,
                    ins=[bin_t.opt()], outs=[bout_t.opt()],
                )
                hf = h_for[lname]
                view = hf[:, 0:KC * d].rearrange("p (c x) -> p c x",
                                                 c=NCORES)
                fi = nc.sync.dma_start(
                    view[:, :, T * w:(T + 1) * w],
                    bout_t[:, :, :].rearrange("c p w -> p c w"))
                bins[(lname, T)] = bi
                fills[(lname, T)] = fi

            def pair_mm(agg, d, lname, p, t, start, stop):
                k0 = CHUNK_ORDER[2 * p]
                lhs = h_for[lname][:, k0 * d:(k0 + 2) * d].rearrange(
                    "p (k f) -> p k f", k=2)
                return nc.tensor.matmul(
                    agg[:, :], lhs,
                    cv[:, 2 * p:2 * p + 2, t * 512:(t + 1) * 512],
                    start=start, stop=stop, perf_mode=DR)

            def bpass_s(d, lname, rev=False):
                """Stream-ordered pass (pairs outer, psum tiles inner):
                adjacency groups are consumed & freed progressively, in
                stream order or reversed."""
                aggs = [aggp.tile([d, 512], F32, tag="agg",
                                  name=f"agg_{lname}_{i}") for i in range(NT)]
                ps = range(KC // 2 - 1, -1, -1) if rev else range(KC // 2)
                for i, p in enumerate(ps):
                    for t in range(NT):
                        pair_mm(aggs[t], d, lname, p, t,
                                start=(i == 0), stop=(i == KC // 2 - 1))
                return aggs

            def bpass_t(d, lname):
                """Tile-sequential pass: yields each completed psum col tile
                so its epilogue + exchange pipeline under the next tiles.
                Sections are chained so the scheduler cannot interleave
                them."""
                prev = [None]
                for t in range(NT):
                    agg = aggp.tile([d, 512], F32, tag="agg",
                                    name=f"agg_{lname}_{t}")
                    for p in range(KC // 2):
                        mm = pair_mm(agg, d, lname, p, t,
                                     start=(p == 0), stop=(p == KC // 2 - 1))
                        if p == 0 and prev[0] is not None:
                            tile.add_dep_helper(mm.ins, prev[0].ins,
                                                sync=True,
                                                reason=f"chain {lname} s{t}")
                    prev[0] = mm
                    yield t, agg

            # ============ L1 local transform: H'1 = dis * (X0 @ W1) ==========
            h1loc = wpool.tile([128, MC * D1], F8, tag="h1loc")
            t1k = []
            for kc in range(4):
                tk = aggp.tile([128, MC * 32], F32, tag="agg", name=f"t1k{kc}")
                t1k.append(tk)
                for m in range(MC):
                    nc.tensor.matmul(
                        tk[:, m * 32:(m + 1) * 32],
                        fqs[kc][:, m * 128:(m + 1) * 128],
                        w1_sb[:, kc * 32:(kc + 1) * 32],
                        start=True, stop=True,
                    )
            for T in range(NT):
                cols = slice(T * MT * 32, (T + 1) * MT * 32)
                s01 = spool.tile([128, MT * 32], F32, tag="t1s",
                                 name=f"s01_{T}")
                nc.vector.tensor_copy(s01[:, :], t1k[0][:, cols])
                for kc in range(1, 4):
                    nc.vector.tensor_tensor(s01[:, :], s01[:, :],
                                            t1k[kc][:, cols], op=AG.add)
                for m in range(T * MT, (T + 1) * MT):
                    col = (m % MT) * 32
                    nc.vector.tensor_scalar_mul(
                        h1loc[:, m * D1:(m + 1) * D1],
                        s01[:, col:col + 32],
                        dispp_sb[:, m:m + 1])
                m_exchange(h1loc, D1, "l1", T)
            tile.add_dep_helper(bden_gs[4].ins, bins[("l1", 2)].ins,
                                sync=True,
                                reason="l1 bins before late bden groups")

            # dis/dinv replicated scale rows (loaded in the quiet window
            # before the bsp stream starts)
            disr_sb = dpool_s.tile([128, NLOC], F16, tag="dsc", name="disr")
            nc.scalar.dma_start(disr_sb[:], dis_repl[:, :])
            dinvr_sb = dpool_s.tile([128, NLOC], F16, tag="dinv", name="dinvr")
            nc.scalar.dma_start(dinvr_sb[:], dinv_repl[:, :])

            # ============ L1 agg + post: x1 = relu(dis*G1 + b1) ==============
            # epilogue + l2-input exchange in REVERSE thirds: l2 consumes its
            # pairs in reverse stream order (so the bsp stream, written in
            # reverse group order, can start as soon as l2 starts).
            aggs = bpass_s(D1, "l1")
            x1p = ppool.tile([32, NLOC], F16, tag="post", name="x1p")
            h2loc = wpool.tile([128, MC * D2], F8, tag="h2loc")
            tp1 = tpp.tile([128, MC * 32], F16, tag="tp16")
            for T in (2, 1, 0):
                sl = slice(T * 512, (T + 1) * 512)
                g1s = spool.tile([32, 512], F32, tag="g1s", name=f"g1s_{T}")
                nc.vector.tensor_tensor(
                    g1s[:, :], aggs[T][:, :], disr_sb[0:32, sl], op=AG.mult)
                x1t = spool.tile([32, 512], F16, tag="x1t", name=f"x1t_{T}")
                nc.scalar.activation(x1t[:, :], g1s[:, :], AF.Relu,
                                     bias=bias_sb[0:32, 0:1])
                nc.vector.tensor_tensor(
                    x1p[:, sl], x1t[:, :], disr_sb[0:32, sl], op=AG.mult)
                for m in range(T * MT, (T + 1) * MT):
                    nc.tensor.transpose(
                        tp1[:, m * 32:(m + 1) * 32],
                        x1p[:, m * 128:(m + 1) * 128], id16_sb[0:32, 0:32])
                o = T * MT * D2
                nc.vector.tensor_copy(h2loc[:, o:o + MT * D2],
                                      tp1[:, o:o + MT * D2])
                m_exchange(h2loc, D2, "l2", T)

            # ============ L2: agg + x2 = relu(dis*G2 @ W12 + b12) ============
            w12_sb = cpool.tile([33, 64], F16, tag="w12")
            nc.scalar.dma_start(w12_sb[:], w12b[:, :])
            dinvpp_sb = cpool.tile([128, MC], F32, tag="dinvpp")
            nc.scalar.dma_start(dinvpp_sb[:], dinv_pp[:, :])
            aggs = bpass_s(D2, "l2", rev=True)
            # bsp overwrites cden groups in REVERSE order as l2's reversed
            # pass frees them.  The last two groups (g1, g0) additionally
            # wait for the l3 fill so the l3 exchange gets the fabric to
            # itself; l3's reversed pass needs g0 last.
            bsp_gs = {}
            for g in range(NGRP - 1, -1, -1):
                bsp_gs[g] = nc.sync.dma_start(
                    cden[:, g * GW:(g + 1) * GW], bsp[g, :, :])
            g2p = ppool.tile([33, NLOC], F16, tag="post", name="g2p")
            nc.vector.memset(g2p[32:33, :], 1.0)
            h3loc = wpool.tile([128, MC * D3], F8, tag="h3loc")
            for T in (2, 1, 0):
                sl = slice(T * 512, (T + 1) * 512)
                nc.vector.tensor_tensor(
                    g2p[0:32, sl], aggs[T][:, :], disr_sb[0:32, sl],
                    op=AG.mult)
                for m in range(T * MT, (T + 1) * MT):
                    xp = wmmp.tile([128, 64], F32, tag="wmm", name=f"x2_{m}")
                    nc.tensor.matmul(xp[:, :], g2p[:, m * 128:(m + 1) * 128],
                                     w12_sb[:, :], start=True, stop=True)
                    nc.vector.tensor_scalar(
                        h3loc[:, m * D3:(m + 1) * D3], xp[:, :],
                        0.0, dinvpp_sb[:, m:m + 1], op0=AG.max, op1=AG.mult)
                m_exchange(h3loc, D3, "l3", T)
            for g in (1, 0):
                tile.add_dep_helper(bsp_gs[g].ins, fills[("l3", 0)].ins,
                                    sync=True,
                                    reason="l3 exchange before last bsp")

            # ============ L3: agg + x3 = relu(dinv*G3 @ W13 + b13) ===========
            w13_sb = cpool.tile([65, 128], F16, tag="w13")
            nc.scalar.dma_start(w13_sb[:], w13b[:, :])
            aggs = bpass_s(D3, "l3", rev=True)
            g3p = ppool.tile([65, NLOC], F16, tag="post", name="g3p")
            nc.vector.memset(g3p[64:65, :], 1.0)
            h4loc = wpool.tile([128, MC * D4], F8, tag="h4loc")
            for T in range(NT):
                sl = slice(T * 512, (T + 1) * 512)
                nc.vector.tensor_tensor(
                    g3p[0:64, sl], aggs[T][:, :], dinvr_sb[0:64, sl],
                    op=AG.mult)
                for m in range(T * MT, (T + 1) * MT):
                    xp = wmmp.tile([128, 128], F32, tag="wmm", name=f"x3_{m}")
                    nc.tensor.matmul(xp[:, :], g3p[:, m * 128:(m + 1) * 128],
                                     w13_sb[:, :], start=True, stop=True)
                    nc.vector.tensor_scalar(
                        h4loc[:, m * D4:(m + 1) * D4], xp[:, :],
                        0.0, dinvpp_sb[:, m:m + 1], op0=AG.max, op1=AG.mult)
                m_exchange(h4loc, D4, "l4", T)

            # ===== L4: agg + x4T = relu(W14^T @ (dinv*G4) + b14) =============
            # ===== L5a: H'5T = dinv * (W2^T @ x4T), transpose, exchange ======
            w14_sb = cpool.tile([128, 128], F16, tag="w14")
            nc.scalar.dma_start(w14_sb[:], w14[:, :])
            w2_sb = cpool.tile([128, CLS], F16, tag="w2")
            nc.scalar.dma_start(w2_sb[:], w2[:, :])
            g4p = ppool.tile([128, NLOC], F16, tag="post", name="g4p")
            x4T = ppool.tile([128, NLOC], F16, tag="post", name="x4T")
            h5T = ppool.tile([32, NLOC], F16, tag="post", name="h5T")
            nc.vector.memset(h5T[0:32, :], 0.0)
            h5loc = wpool.tile([128, MC * D5], F8, tag="h5loc")
            tp5 = tpp.tile([128, MC * 32], F16, tag="tp16")
            for T, agg in bpass_t(D4, "l4"):
                sl = slice(T * 512, (T + 1) * 512)
                nc.vector.tensor_tensor(
                    g4p[:, sl], agg[:, :], dinvr_sb[:, sl], op=AG.mult)
                x4p = wmmp.tile([128, 512], F32, tag="wmm", name=f"x4_{T}")
                nc.tensor.matmul(x4p[:, :], w14_sb[:, :], g4p[:, sl],
                                 start=True, stop=True)
                nc.scalar.activation(x4T[:, sl], x4p[:, :], AF.Relu,
                                     bias=bias_sb[:, 1:2])
                t5 = wmmp.tile([CLS, 512], F32, tag="wmm", name=f"t5_{T}")
                nc.tensor.matmul(t5[:, :], w2_sb[:, :], x4T[:, sl],
                                 start=True, stop=True)
                nc.vector.tensor_tensor(
                    h5T[0:CLS, sl], t5[:, :], dinvr_sb[0:CLS, sl],
                    op=AG.mult)
                for m in range(T * MT, (T + 1) * MT):
                    nc.tensor.transpose(
                        tp5[:, m * 32:(m + 1) * 32],
                        h5T[:, m * 128:(m + 1) * 128], id16_sb[0:32, 0:32])
                o = T * MT * D5
                nc.vector.tensor_copy(h5loc[:, o:o + MT * D5],
                                      tp5[:, o:o + MT * D5])
                m_exchange(h5loc, D5, "l5", T)

            # ============ L5b: agg + z = dinv*G5 + b2, log_softmax ===========
            id32_sb = cpool.tile([128, 128], F32, tag="id32")
            nc.scalar.dma_start(id32_sb[:], ident32[:, :])
            zt = wpool.tile([32, NLOC], F32, tag="zt")
            nc.vector.memset(zt[0:32, :], 0.0)
            ztp = tpp.tile([128, MC * 32], F32, tag="tp32")
            outsb = wpool.tile([128, MC * CLS], F32, tag="outsb")
            nmt = wpool.tile([128, MC], F32, tag="nmt")
            et = wpool.tile([128, MC * CLS], F32, tag="et")
            st = wpool.tile([128, MC], F32, tag="st")
            lst = wpool.tile([128, MC], F32, tag="lst")
            zs = wpool.tile([128, MC * CLS], F32, tag="zs")
            ztpv = ztp[:].rearrange("p (m f) -> p m f", m=MC)
            etv = et[:].rearrange("p (m f) -> p m f", m=MC)
            for T, agg in bpass_t(D5, "l5"):
                sl = slice(T * 512, (T + 1) * 512)
                ms = slice(T * MT, (T + 1) * MT)
                nc.vector.tensor_tensor(
                    zt[0:CLS, sl], agg[0:CLS, :], dinvr_sb[0:CLS, sl],
                    op=AG.mult)
                nc.vector.tensor_scalar_add(
                    zt[0:CLS, sl], zt[0:CLS, sl], bias_sb[0:CLS, 2:3])
                for m in range(T * MT, (T + 1) * MT):
                    nc.tensor.transpose(
                        ztp[:, m * 32:(m + 1) * 32],
                        zt[:, m * 128:(m + 1) * 128], id32_sb[0:32, 0:32])
                nc.vector.reduce_max(
                    nmt[:, ms], ztpv[:, ms, 0:CLS],
                    axis=mybir.AxisListType.X, negate=True)
                for m in range(T * MT, (T + 1) * MT):
                    nc.vector.tensor_scalar_add(
                        zs[:, m * CLS:(m + 1) * CLS],
                        ztp[:, m * 32: m * 32 + CLS], nmt[:, m:m + 1])
                nc.scalar.activation(et[:, T * MT * CLS:(T + 1) * MT * CLS],
                                     zs[:, T * MT * CLS:(T + 1) * MT * CLS],
                                     AF.Exp)
                nc.vector.reduce_sum(
                    st[:, ms], etv[:, ms, :], axis=mybir.AxisListType.X)
                nc.scalar.activation(lst[:, ms], st[:, ms], AF.Ln)
                for m in range(T * MT, (T + 1) * MT):
                    nc.vector.tensor_scalar(
                        outsb[:, m * CLS:(m + 1) * CLS],
                        ztp[:, m * 32: m * 32 + CLS],
                        nmt[:, m:m + 1], lst[:, m:m + 1],
                        op0=AG.add, op1=AG.subtract)
                nc.scalar.dma_start(
                    out.ap()[:, T * MT * CLS:(T + 1) * MT * CLS],
                    outsb[:, T * MT * CLS:(T + 1) * MT * CLS])

    nc.compile()
    return nc


# ---------------------------------------------------------------------------
# host-side preprocessing
# ---------------------------------------------------------------------------

def _preprocess(node_feats, edge_index, W1, b1, W12, b12, W13, b13, W14, b14,
                W2, b2):
    src = np.asarray(edge_index[0], dtype=np.int64)
    dst = np.asarray(edge_index[1], dtype=np.int64)

    # dense-path matrix: B[i,j] = #edges(i->j) offdiag, diag forced to 1
    Bden = np.zeros(NP * NP, dtype=np.uint8)
    np.add.at(Bden, src * NP + dst, 1)
    Bden = Bden.reshape(NP, NP)
    idx = np.arange(N)
    Bden[idx, idx] = 1
    deg_den = Bden[:N].sum(axis=1, dtype=np.int64).astype(np.float64)
    dis = np.zeros(NP, dtype=np.float64)
    dis[:N] = np.maximum(deg_den, 1.0) ** -0.5
    dis[N:] = 1.0

    # sparse-path matrix: Bsp[t,s] = #edges(s->t) + I
    Bsp = np.zeros(NP * NP, dtype=np.uint8)
    np.add.at(Bsp, dst * NP + src, 1)
    Bsp = Bsp.reshape(NP, NP)
    Bsp[idx, idx] += 1
    deg_sp = Bsp[:N].sum(axis=1, dtype=np.int64).astype(np.float64)
    dinv = np.zeros(NP, dtype=np.float64)
    dinv[:N] = np.where(deg_sp > 0, deg_sp.astype(np.float64) ** -0.5, 0.0)

    x0 = np.zeros((NP, F_IN), dtype=np.float32)
    x0[:N] = np.asarray(node_feats, dtype=np.float32)

    def pp(vec, c):
        loc = vec[c * NLOC:(c + 1) * NLOC].astype(np.float32)
        return np.ascontiguousarray(loc.reshape(MC, 128).T)

    def repl(vec, c):
        loc = vec[c * NLOC:(c + 1) * NLOC].astype(NP_F16)
        return np.ascontiguousarray(np.broadcast_to(loc[None, :], (128, NLOC)))

    def pack_b(B, rows):
        # [s, t_local] chunked over s, in CHUNK_ORDER, group-major so each
        # DMA group is one fully contiguous block of DRAM.
        bt = B[rows].T.reshape(KC, 128, NLOC)[CHUNK_ORDER]
        bt = bt.reshape(NGRP, GC, 128, NLOC).transpose(0, 2, 1, 3)
        return np.ascontiguousarray(bt.reshape(NGRP, 128, GW)).astype(NP_F8)

    w12b = np.concatenate([np.asarray(W12, np.float32),
                           np.asarray(b12, np.float32)[None, :]], axis=0)
    w13b = np.concatenate([np.asarray(W13, np.float32),
                           np.asarray(b13, np.float32)[None, :]], axis=0)
    biases_pp = np.zeros((128, 3), dtype=np.float32)
    biases_pp[:32, 0] = np.asarray(b1, np.float32)
    biases_pp[:, 1] = np.asarray(b14, np.float32)
    biases_pp[:CLS, 2] = np.asarray(b2, np.float32)

    in_maps = []
    for c in range(NCORES):
        rows = slice(c * NLOC, (c + 1) * NLOC)
        featT_c = np.ascontiguousarray(x0[rows].T).reshape(4, 128, NLOC)
        in_maps.append({
            "bden": pack_b(Bden, rows),
            "bsp": pack_b(Bsp, rows),
            "featT": featT_c.astype(NP_F16),
            "w1": np.asarray(W1, np.float32).reshape(4, 128, 32).astype(NP_F16),
            "w12b": w12b.astype(NP_F16),
            "w13b": w13b.astype(NP_F16),
            "w14": np.asarray(W14, np.float32).astype(NP_F16),
            "w2": np.asarray(W2, np.float32).astype(NP_F16),
            "biases_pp": biases_pp,
            "dis_repl": repl(dis, c),
            "dinv_repl": repl(dinv, c),
            "dis_pp": pp(dis, c),
            "dinv_pp": pp(dinv, c),
            "ident16": np.eye(128, dtype=NP_F16),
            "ident32": np.eye(128, dtype=np.float32),
        })
    return in_maps


def kernel(node_feats, edge_index, W1, b1, W12, b12, W13, b13, W14, b14, W2,
           b2):
    in_maps = _preprocess(node_feats, edge_index, W1, b1, W12, b12, W13, b13,
                          W14, b14, W2, b2)
    if "nc" not in _cached:
        _cached["nc"] = _build_program()
    nc = _cached["nc"]
    trace = bool(int(os.environ.get("KERNEL_TRACE", "0")))
    tca = os.environ.get("TRACE_ALL_CORES")
    res = run_bass_kernel_spmd(nc, in_maps, core_ids=list(range(NCORES)),
                               trace=trace,
                               trace_cores=(list(range(NCORES)) if tca
                                            else None))
    _cached["last_result"] = res
    outs = []
    for c in range(NCORES):
        o = np.asarray(res.results[c]["out"])     # [128, MC*CLS]
        outs.append(o.reshape(128, MC, CLS).transpose(1, 0, 2)
                     .reshape(NLOC, CLS))
    return np.concatenate(outs, axis=0)[:N].astype(np.float32)
